# revision 49
# baseline (speedup 1.0000x reference)
"""Trainium2 Bass kernel for nn_Blur: 4x4 FIR depthwise blur with pad (2,1).

out[n,c,i,j] = sum_{a,b} K[a,b] * x[n,c, i+1-a, j+1-b]   (zero-padded)

Strategy (8 NeuronCores, pure data parallelism over the 8192 (n,c) slices):
  - fp16 end-to-end on device (host converts): halves HBM traffic vs fp32.
    Quantization error ~5e-4 relative, far under the 2e-2 gate.
  - w-parity interleaved layout, partition p = 64*(w%2) + h; free dim packs
    each slice as 32 w-blocks of 2. The 16-tap conv is THREE PSUM-accumulated
    matmuls (free-dim block shifts d in {-1,0,+1}):
    lhsT_d[(jp_in,u),(jp_out,i)] = K[i-u+1, jp_out-jp_in+1-2d].
    Group-outer / d-inner order: each 512-col group's PSUM completes after
    its 3 matmuls, so copies+stores drain steadily instead of in bursts.
  - DMA: only two HW DGE rings exist (sync=qSP, scalar=qAct). Ring
    throughput is PACKET-count limited early on (~11 GB/s/engine at 1KB
    rows, ~26 at 4KB), so the FIRST transfer fuses weights+2 groups into
    one 3KB-row DMA; everything else moves in 4KB rows.
  - Startup: the HAM clock gate needs ~4.2us of CONTIGUOUS observed PE
    activity to open (1.2 -> 2.4 GHz) and a >0.5us idle gap resets the
    accumulator. Junk matmuls on an UNINITIALIZED tile (no memset, no DMA
    dependency -- garbage values are discarded via warm_out) start at the
    tensor engine's first post-preamble slot and bridge into the first
    real matmul with no gap.
  - Drain: tile 15 is split 2+1+1; the final two single-group chunks copy
    on scalar and vector in parallel and store down both rings at once.
"""

import sys
import types

import numpy as np

import concourse.bacc as bacc
import concourse.mybir as mybir
from concourse.alu_op_type import AluOpType
from concourse.tile import TileContext
from concourse.bass_utils import run_bass_kernel_spmd


def _install_ntff_hook():
    """Best-effort shim: this image's antenv lacks axon_hooks, which the
    trace=True path of run_bass_kernel_spmd imports. Harmless if unused."""
    if "antenv.axon_hooks" in sys.modules:
        return
    try:
        sys.path.insert(0, "/root/.axon_site")
        from trn_agent_boot.trn_boot import _ntff_profile_via_ctypes

        hook = _ntff_profile_via_ctypes("/opt/axon/libaxon_pjrt.so")
        mod = types.ModuleType("antenv.axon_hooks")
        mod.get_axon_ntff_profile_hook = lambda: hook
        mod.set_axon_ntff_profile_hook = lambda h: None
        sys.modules["antenv.axon_hooks"] = mod
    except Exception:
        pass


_install_ntff_hook()

N_CORES = 8
B, C, H, W = 32, 256, 64, 64
NSLICES = B * C                      # 8192
SLICES_PER_CORE = NSLICES // N_CORES  # 1024
TILE_SLICES = 64                     # slices per full SBUF tile
JB = W // 2                          # 32 w-blocks of 2 per slice
FREE = TILE_SLICES * JB              # 2048: NO padding (edge-skip matmuls)
GQ = 16                              # slices per PSUM group (N = 16*32 = 512)
GF = GQ * JB                         # free columns per group = 512
WP = W + 3                           # offload path: 2 left + 1 right zero
SG = TILE_SLICES // 2                # offload path: s-groups per member
F16 = mybir.dt.float16
F32 = mybir.dt.float32

# Separable offload of whole tiles: GpSimd does the W-conv pair sums
# (t1 = x<<0 + x<<3, t2 = x<<1 + x<<2, ~3.6us each at the measured
# 73 G elem/s), and the PE's h-band pass at the END of its stream fuses
# y = (k1/16)^T t1 + (3*k1/16)^T t2 as two accumulating matmuls -- no DVE
# work at all. Each offloaded tile trades 6144 main-path PE cycles for
# 4096 back-phase cycles (-0.85us/tile). xof tiles load on the SCALAR
# ring, which is empty early -- the sync ring's load pacing (which just
# barely outruns the PE) is untouched.
# Offload measured NET NEGATIVE in three schedule variants (rounds 3-5):
# the load stream outruns the PE by only ~10% once stores ramp, so moving
# the xof bytes anywhere early starves the PE mid-stream (HAM gate drops,
# half-clock), and moving them late starves GpSimd. Keep it off.
OFFLOAD = ()
# Junk bridge runs PAST the clock-gate opening (~12.05us): cores whose
# first data lands early lose only the half-clock sliver of real work
# before the gate, while cores with late data (12.5-12.9us observed across
# the 8 cores) avoid a ~6-10us HAM-reset penalty. Max-over-cores is what
# counts, so robustness wins.
WARMUP_MMS = 46                      # 128-col junk matmuls: ~7.6us -> ~12.6us

_NC_CACHE = {}


def _build_wmat(K: np.ndarray) -> np.ndarray:
    """[128, 4*128] fp16: lhsT stack [d=0, d=-1, d=+1, h-band k1/16]."""
    K = np.asarray(K, np.float32)
    wmat = np.zeros((4, 128, 128), np.float32)
    for di, d in enumerate((0, -1, 1)):
        L = wmat[di]
        for jpi in range(2):
            for jpo in range(2):
                b = jpo - jpi + 1 - 2 * d
                if not (0 <= b < 4):
                    continue
                for i in range(H):
                    for a in range(4):
                        u = i + 1 - a
                        if 0 <= u < H:
                            L[64 * jpi + u, 64 * jpo + i] += K[a, b]
    # h-bands for the separable path: lhsT[u+64m, i+64m] = s*k1[i-u+1]/16
    # slot 3 applies to t1 (s=1), slot 4 to t2 (s=3): the PE fuses the
    # W-conv's final 3*t2 + t1 into its PSUM accumulation.
    wmat = np.concatenate([wmat, np.zeros((1, 128, 128), np.float32)])
    k1 = np.array([1.0, 3.0, 3.0, 1.0], np.float32) / 16.0
    T = np.zeros((H, H), np.float32)
    for i in range(H):
        for a in range(4):
            u = i + 1 - a
            if 0 <= u < H:
                T[u, i] += k1[a]
    for sl, s in ((3, 1.0), (4, 3.0)):
        wmat[sl, :H, :H] = s * T
        wmat[sl, H:, H:] = s * T
    # [d, k, m] -> [k, (d m)] so the DMA is one contiguous run per partition
    return np.ascontiguousarray(
        wmat.transpose(1, 0, 2).reshape(128, 5 * 128)
    ).astype(np.float16)


def _build_nc(slices_per_core: int = SLICES_PER_CORE):
    ntiles = slices_per_core // TILE_SLICES
    nc = bacc.Bacc("TRN2", target_bir_lowering=False, debug=False)
    x = nc.dram_tensor(
        "x", [ntiles, 128, FREE], F16, kind="ExternalInput"
    ).ap()
    wm = nc.dram_tensor("w", [128, 5 * 128], F16, kind="ExternalInput").ap()
    xo = (
        nc.dram_tensor(
            "xo", [len(OFFLOAD), 128, SG * WP], F16, kind="ExternalInput"
        ).ap()
        if OFFLOAD
        else None
    )
    y = nc.dram_tensor(
        "y", [ntiles, 128, TILE_SLICES * JB], F16, kind="ExternalOutput"
    ).ap()
    # sink for the PE warm-up matmuls (kept alive so DCE can't drop them)
    warm_out = nc.dram_tensor("warm", [128, 4], F32, kind="ExternalOutput").ap()

    # main-path chunk list: tile 0 split 2+2 (2KB-row first transfer is
    # the empirically fastest first landing -- 1KB and 3KB rows both lose
    # to the ring's cold-start shape); tile 15 split 2+2 so the last two
    # stores are 2KB-row 2-group transfers down both rings in parallel.
    # tile 0 moves as 2-group chunks; tiles 1+ as full 4-group (4KB-row)
    # chunks -- splitting tiles 1-2 was measured SLOWER (2KB rows halve the
    # cold ring's per-engine rate; the PE gap it opened reset the HAM gate)
    chunks = [(0, 0, 2), (0, 2, 2)]
    chunks += [(t, 0, 4) for t in range(1, ntiles - 1) if t not in OFFLOAD]
    if (ntiles - 1) not in OFFLOAD:
        # tile 15 split 2+1+1: the final two single-group stores go down
        # different rings in parallel, each right after its own copy
        chunks += [
            (ntiles - 1, 0, 2),
            (ntiles - 1, 2, 1),
            (ntiles - 1, 3, 1),
        ]
    last = len(chunks) - 1

    with TileContext(nc) as tc:
        with (
            tc.tile_pool(name="wpool", bufs=1) as wpool,
            tc.tile_pool(name="xpool", bufs=12) as xpool,
            tc.tile_pool(name="vpool", bufs=4) as vpool,
            tc.tile_pool(name="opool", bufs=6) as opool,
            tc.tile_pool(name="pspool", bufs=8, space="PSUM") as pspool,
        ):
            # weight tile: rides the SP ring right behind tile 0's first
            # two groups (the baseline-measured fastest start: both sems
            # land ~10.7us).
            wsb = wpool.tile([128, 5, 128], F16, name="wsb")

            def wap(di):
                return wsb[:, di, :]

            oi = {t: i for i, t in enumerate(OFFLOAD)}
            ofst = {}

            def offload_front(t):
                """Load xof (sync ring, AFTER the first few x tiles -- a
                front-loaded xof steals cold-phase DMA capacity from the
                PE-pacing x loads) + GpSimd W-conv pair sums."""
                xoft = xpool.tile([128, SG, WP], F16, name="xof")
                nc.sync.dma_start(xoft[:], xo[oi[t]])
                t1 = vpool.tile([128, SG, W], F16, name="t1")
                t2 = vpool.tile([128, SG, W], F16, name="t2")
                nc.gpsimd.tensor_tensor(
                    t1[:], xoft[:, :, 0:W], xoft[:, :, 3 : 3 + W],
                    AluOpType.add,
                )
                nc.gpsimd.tensor_tensor(
                    t2[:], xoft[:, :, 1 : 1 + W], xoft[:, :, 2 : 2 + W],
                    AluOpType.add,
                )
                ofst[t] = (t1, t2)

            # HAM warm-up: a tiny [128,128] memset on DVE (~150ns at its
            # first post-preamble slot) unblocks a run of 128-col junk
            # matmuls that keep the PE busy from ~7.3us until the t0 DMA
            # lands (~9.9us) -- the clock-gate accumulator never resets.
            wjunk = wpool.tile([128, 128], F16, name="wjunk")
            nc.vector.memset(wjunk[:], 0.0)
            wscratch = wpool.tile([128, 4], F32, name="wscratch")
            wps = pspool.tile([128, 128], F32, name="wps", tag="ps")
            for r in range(WARMUP_MMS):
                nc.tensor.matmul(
                    wps[:],
                    wjunk[:],
                    wjunk[:],
                    start=(r == 0),
                    stop=(r == WARMUP_MMS - 1),
                )
            # warm store on the gpsimd software queue: scalar's instruction
            # stream must stay free for the x1 load issue (this store waits
            # on the junk chain and would block it until ~12.5us)
            nc.vector.tensor_copy(wscratch[:], wps[:, 0:4])
            nc.gpsimd.dma_start(warm_out, wscratch[:])

            ncopy = 0

            def offload_back(t, tail=False):
                """Fused h-band PE passes + copies + store, at stream end:
                ps = T^T t1 + (3T)^T t2 completes the separable blur."""
                t1, t2 = ofst[t]
                ot = opool.tile([128, SG, W], F16, name="ot")
                for q in range(4):
                    sl = slice(8 * q, 8 * (q + 1))
                    ps = pspool.tile([128, GQ * JB], F32, name="ps")
                    nc.tensor.matmul(
                        ps[:], wap(3), t1[:, sl, :], start=True, stop=False
                    )
                    nc.tensor.matmul(
                        ps[:], wap(4), t2[:, sl, :], start=False, stop=True
                    )
                    dst = ot[:, sl, :]
                    if q % 2 == 0:
                        nc.scalar.copy(dst, ps[:])
                    else:
                        nc.vector.tensor_copy(dst, ps[:])
                    if tail and q == 1:
                        nc.scalar.dma_start(
                            y[t][:, 0 : 2 * GQ * JB], ot[:, 0:16, :]
                        )
                if tail:
                    nc.sync.dma_start(
                        y[t][:, 2 * GQ * JB :], ot[:, 16:32, :]
                    )
                else:
                    nc.scalar.dma_start(y[t], ot[:])

            # tile 1 loads via the scalar ring, issued up-front (the scalar
            # engine's in-loop program position trails its copy stream, so
            # a late issue point makes loads just-in-time -- measured as a
            # 5.6us PE stall when tiles 12-14 were moved this way). One
            # 0.5MB transfer in parallel with the sync ring's cold ramp is
            # exactly the transfer the PE needs next; more would starve the
            # ramp (round-4 lesson).
            x1t = xpool.tile([128, 4 * GQ, JB], F16, name="xe")
            nc.scalar.dma_start(x1t[:], x[1])

            for ci, (dt, g0, ng) in enumerate(chunks):
                if dt == 1:
                    xt = x1t
                else:
                    xt = xpool.tile([128, ng * GQ, JB], F16, name="xt")
                    nc.sync.dma_start(
                        xt[:], x[dt][:, g0 * GF : (g0 + ng) * GF]
                    )
                if ci == 0:
                    # weights ride the SP ring second: land with chunk 0
                    nc.sync.dma_start(wsb[:], wm)
                grp = lambda g: xt[:, GQ * g : GQ * (g + 1), :]
                grpl = lambda g: xt[:, GQ * g : GQ * (g + 1), 0 : JB - 1]
                grpr = lambda g: xt[:, GQ * g : GQ * (g + 1), 1:JB]

                ot = opool.tile([128, ng * GQ, JB], F16, name="ot")
                # group-outer, d-inner: group q's PSUM is complete after its
                # own 3 matmuls; its copy runs while the PE streams q+1.
                # No padding: d=-1 skips output col jb=0, d=+1 skips jb=31.
                tailc = (not OFFLOAD) and ci >= last - 2
                for q in range(ng):
                    ps = pspool.tile([128, GQ, JB], F32, name="ps")
                    nc.tensor.matmul(
                        ps[:], wap(0), grp(q), start=True, stop=False
                    )
                    nc.tensor.matmul(
                        ps[:, :, 1:JB], wap(1), grpl(q), start=False, stop=False
                    )
                    nc.tensor.matmul(
                        ps[:, :, 0 : JB - 1], wap(2), grpr(q),
                        start=False, stop=True,
                    )
                    dst = ot[:, GQ * q : GQ * (q + 1), :]
                    if tailc:
                        # drain chunks: pin copy engines so the final two
                        # single-group chunks drain on scalar and vector
                        # (and then both rings) in parallel
                        if ci == last or (ci == last - 2 and q == 1):
                            nc.vector.tensor_copy(dst, ps[:])
                        else:
                            nc.scalar.copy(dst, ps[:])
                        continue
                    # alternate copy engine: DVE and ACT share the load
                    if ncopy % 2 == 0:
                        nc.vector.tensor_copy(dst, ps[:])
                    else:
                        nc.scalar.copy(dst, ps[:])
                    ncopy += 1
                ylo = g0 * GQ * JB
                if not tailc:
                    nc.scalar.dma_start(
                        y[dt][:, ylo : ylo + ng * GQ * JB], ot[:]
                    )
                else:
                    # drain stores: (15,0,2) and (15,3,1) ride the sync
                    # ring so scalar's chain is just copy(15b)+store(15b) --
                    # the two final stores overlap on different rings.
                    # (All-scalar draining was measured ~1.3us slower.)
                    eng = nc.scalar if ci == last - 1 else nc.sync
                    eng.dma_start(
                        y[dt][:, ylo : ylo + ng * GQ * JB], ot[:]
                    )
                # xof loads ride the sync ring behind x3..x6: early enough
                # for GpSimd's 28.8us of adds, late enough not to starve
                # the PE's own load stream during the cold ramp
                if OFFLOAD and ci - 4 in range(len(OFFLOAD)):
                    offload_front(OFFLOAD[ci - 4])

            # offloaded tiles' h-band passes close the PE stream
            for k, t in enumerate(OFFLOAD):
                offload_back(t, tail=(k == len(OFFLOAD) - 1))

    nc.compile()
    return nc


def get_nc(slices_per_core: int = SLICES_PER_CORE):
    if slices_per_core not in _NC_CACHE:
        _NC_CACHE[slices_per_core] = _build_nc(slices_per_core)
    return _NC_CACHE[slices_per_core]


def _pack_input(xs: np.ndarray):
    """[S, H, W] fp16 -> main tiles [S/64, 128, FREE] + offload tiles."""
    s = xs.shape[0]
    ntiles = s // TILE_SLICES
    v = np.empty((ntiles, 2, H, TILE_SLICES, JB), np.float16)
    xt = xs.reshape(ntiles, TILE_SLICES, H, W)
    v[:, 0] = xt[:, :, :, 0::2].transpose(0, 2, 1, 3)
    v[:, 1] = xt[:, :, :, 1::2].transpose(0, 2, 1, 3)
    xmain = np.ascontiguousarray(v.reshape(ntiles, 128, FREE))
    if not OFFLOAD:
        return xmain, None
    # offload tiles: partition (m, h), free (sg, w) with w zero-padded to 67
    xofs = np.zeros((len(OFFLOAD), 128, SG * WP), np.float16)
    for i, t in enumerate(OFFLOAD):
        xp = np.zeros((TILE_SLICES, H, WP), np.float16)
        xp[:, :, 2 : 2 + W] = xt[t]
        # (sg, m, h, w) -> (m, h, sg, w)
        xofs[i] = (
            xp.reshape(SG, 2, H, WP)
            .transpose(1, 2, 0, 3)
            .reshape(128, SG * WP)
        )
    return xmain, xofs


def _unpack_output(yp: np.ndarray) -> np.ndarray:
    """[S/64, 128, 64*JB] fp16 -> [S, H, W] fp16 (mixed per-tile layouts)."""
    ntiles = yp.shape[0]
    out = np.empty((ntiles, TILE_SLICES, H, W), np.float16)
    # main path: [jp, i, s, jb]
    v = yp.reshape(ntiles, 2, H, TILE_SLICES, JB)
    out[:, :, :, 0::2] = v[:, 0].transpose(0, 2, 1, 3)
    out[:, :, :, 1::2] = v[:, 1].transpose(0, 2, 1, 3)
    # offload path: [m, i, sg, w]
    for t in OFFLOAD:
        if t < ntiles:
            vo = yp[t].reshape(2, H, SG, W)
            out[t] = vo.transpose(2, 0, 1, 3).reshape(TILE_SLICES, H, W)
    return out.reshape(ntiles * TILE_SLICES, H, W)


def kernel(x: np.ndarray, kernel: np.ndarray, _trace: bool = False, **_tkw):
    xh = np.asarray(x).astype(np.float16)
    wmat = _build_wmat(kernel)
    b, c, h, w = x.shape
    xs = xh.reshape(b * c, h, w)
    spc = (b * c) // N_CORES
    nc = get_nc(spc)
    in_maps = []
    for k in range(N_CORES):
        xmain, xofs = _pack_input(xs[k * spc : (k + 1) * spc])
        m = {"x": xmain, "w": wmat}
        if xofs is not None:
            m["xo"] = xofs
        in_maps.append(m)
    res = run_bass_kernel_spmd(
        nc, in_maps, list(range(N_CORES)), trace=_trace, **_tkw
    )
    out = np.concatenate(
        [_unpack_output(res.results[k]["y"]) for k in range(N_CORES)], axis=0
    )
    result = out.reshape(b, c, h, w).astype(np.float32)
    if _trace:
        return result, res
    return result


# revision 50
# speedup vs baseline: 1.0637x; 1.0637x over previous
"""Trainium2 Bass kernel for nn_Blur: 4x4 FIR depthwise blur with pad (2,1).

out[n,c,i,j] = sum_{a,b} K[a,b] * x[n,c, i+1-a, j+1-b]   (zero-padded)

Strategy (8 NeuronCores, pure data parallelism over the 8192 (n,c) slices):
  - fp16 end-to-end on device (host converts): halves HBM traffic vs fp32.
    Quantization error ~5e-4 relative, far under the 2e-2 gate.
  - w-parity interleaved layout, partition p = 64*(w%2) + h; free dim packs
    each slice as 32 w-blocks of 2. The 16-tap conv is THREE PSUM-accumulated
    matmuls (free-dim block shifts d in {-1,0,+1}):
    lhsT_d[(jp_in,u),(jp_out,i)] = K[i-u+1, jp_out-jp_in+1-2d].
    Group-outer / d-inner order: each 512-col group's PSUM completes after
    its 3 matmuls, so copies+stores drain steadily instead of in bursts.
  - DMA: only two HW DGE rings exist (sync=qSP, scalar=qAct). Ring
    throughput is PACKET-count limited early on (~11 GB/s/engine at 1KB
    rows, ~26 at 4KB), so the FIRST transfer fuses weights+2 groups into
    one 3KB-row DMA; everything else moves in 4KB rows.
  - Startup: the HAM clock gate needs ~4.2us of CONTIGUOUS observed PE
    activity to open (1.2 -> 2.4 GHz) and a >0.5us idle gap resets the
    accumulator. Junk matmuls on an UNINITIALIZED tile (no memset, no DMA
    dependency -- garbage values are discarded via warm_out) start at the
    tensor engine's first post-preamble slot and bridge into the first
    real matmul with no gap.
  - Drain: tile 15 is split 2+1+1; the final two single-group chunks copy
    on scalar and vector in parallel and store down both rings at once.
"""

import sys
import types

import numpy as np

import concourse.bacc as bacc
import concourse.mybir as mybir
from concourse.alu_op_type import AluOpType
from concourse.tile import TileContext
from concourse.bass_utils import run_bass_kernel_spmd


def _install_ntff_hook():
    """Best-effort shim: this image's antenv lacks axon_hooks, which the
    trace=True path of run_bass_kernel_spmd imports. Harmless if unused."""
    if "antenv.axon_hooks" in sys.modules:
        return
    try:
        sys.path.insert(0, "/root/.axon_site")
        from trn_agent_boot.trn_boot import _ntff_profile_via_ctypes

        hook = _ntff_profile_via_ctypes("/opt/axon/libaxon_pjrt.so")
        mod = types.ModuleType("antenv.axon_hooks")
        mod.get_axon_ntff_profile_hook = lambda: hook
        mod.set_axon_ntff_profile_hook = lambda h: None
        sys.modules["antenv.axon_hooks"] = mod
    except Exception:
        pass


_install_ntff_hook()

N_CORES = 8
B, C, H, W = 32, 256, 64, 64
NSLICES = B * C                      # 8192
SLICES_PER_CORE = NSLICES // N_CORES  # 1024
TILE_SLICES = 64                     # slices per full SBUF tile
JB = W // 2                          # 32 w-blocks of 2 per slice
FREE = TILE_SLICES * JB              # 2048: NO padding (edge-skip matmuls)
GQ = 16                              # slices per PSUM group (N = 16*32 = 512)
GF = GQ * JB                         # free columns per group = 512
WP = W + 3                           # offload path: 2 left + 1 right zero
SG = TILE_SLICES // 2                # offload path: s-groups per member
F16 = mybir.dt.float16
F32 = mybir.dt.float32

# Separable offload of whole tiles: GpSimd does the W-conv pair sums
# (t1 = x<<0 + x<<3, t2 = x<<1 + x<<2, ~3.6us each at the measured
# 73 G elem/s), and the PE's h-band pass at the END of its stream fuses
# y = (k1/16)^T t1 + (3*k1/16)^T t2 as two accumulating matmuls -- no DVE
# work at all. Each offloaded tile trades 6144 main-path PE cycles for
# 4096 back-phase cycles (-0.85us/tile). xof tiles load on the SCALAR
# ring, which is empty early -- the sync ring's load pacing (which just
# barely outruns the PE) is untouched.
# Offload measured NET NEGATIVE in three schedule variants (rounds 3-5):
# the load stream outruns the PE by only ~10% once stores ramp, so moving
# the xof bytes anywhere early starves the PE mid-stream (HAM gate drops,
# half-clock), and moving them late starves GpSimd. Keep it off.
OFFLOAD = ()
# Junk bridge runs PAST the clock-gate opening (~12.05us): cores whose
# first data lands early lose only the half-clock sliver of real work
# before the gate, while cores with late data (12.5-12.9us observed across
# the 8 cores) avoid a ~6-10us HAM-reset penalty. Max-over-cores is what
# counts, so robustness wins.
WARMUP_MMS = 50                      # 128-col junk matmuls: ~7.6us -> ~13.0us

_NC_CACHE = {}


def _build_wmat(K: np.ndarray) -> np.ndarray:
    """[128, 4*128] fp16: lhsT stack [d=0, d=-1, d=+1, h-band k1/16]."""
    K = np.asarray(K, np.float32)
    wmat = np.zeros((4, 128, 128), np.float32)
    for di, d in enumerate((0, -1, 1)):
        L = wmat[di]
        for jpi in range(2):
            for jpo in range(2):
                b = jpo - jpi + 1 - 2 * d
                if not (0 <= b < 4):
                    continue
                for i in range(H):
                    for a in range(4):
                        u = i + 1 - a
                        if 0 <= u < H:
                            L[64 * jpi + u, 64 * jpo + i] += K[a, b]
    # h-bands for the separable path: lhsT[u+64m, i+64m] = s*k1[i-u+1]/16
    # slot 3 applies to t1 (s=1), slot 4 to t2 (s=3): the PE fuses the
    # W-conv's final 3*t2 + t1 into its PSUM accumulation.
    wmat = np.concatenate([wmat, np.zeros((1, 128, 128), np.float32)])
    k1 = np.array([1.0, 3.0, 3.0, 1.0], np.float32) / 16.0
    T = np.zeros((H, H), np.float32)
    for i in range(H):
        for a in range(4):
            u = i + 1 - a
            if 0 <= u < H:
                T[u, i] += k1[a]
    for sl, s in ((3, 1.0), (4, 3.0)):
        wmat[sl, :H, :H] = s * T
        wmat[sl, H:, H:] = s * T
    # [d, k, m] -> [k, (d m)] so the DMA is one contiguous run per partition
    return np.ascontiguousarray(
        wmat.transpose(1, 0, 2).reshape(128, 5 * 128)
    ).astype(np.float16)


def _build_nc(slices_per_core: int = SLICES_PER_CORE):
    ntiles = slices_per_core // TILE_SLICES
    nc = bacc.Bacc("TRN2", target_bir_lowering=False, debug=False)
    x = nc.dram_tensor(
        "x", [ntiles, 128, FREE], F16, kind="ExternalInput"
    ).ap()
    wm = nc.dram_tensor("w", [128, 5 * 128], F16, kind="ExternalInput").ap()
    xo = (
        nc.dram_tensor(
            "xo", [len(OFFLOAD), 128, SG * WP], F16, kind="ExternalInput"
        ).ap()
        if OFFLOAD
        else None
    )
    y = nc.dram_tensor(
        "y", [ntiles, 128, TILE_SLICES * JB], F16, kind="ExternalOutput"
    ).ap()
    # sink for the PE warm-up matmuls (kept alive so DCE can't drop them)
    warm_out = nc.dram_tensor("warm", [128, 4], F32, kind="ExternalOutput").ap()

    # main-path chunk list: tile 0 split 2+2 (2KB-row first transfer is
    # the empirically fastest first landing -- 1KB and 3KB rows both lose
    # to the ring's cold-start shape); tile 15 split 2+2 so the last two
    # stores are 2KB-row 2-group transfers down both rings in parallel.
    # tile 0 moves as 2-group chunks; tiles 1+ as full 4-group (4KB-row)
    # chunks -- splitting tiles 1-2 was measured SLOWER (2KB rows halve the
    # cold ring's per-engine rate; the PE gap it opened reset the HAM gate)
    chunks = [(0, 0, 2), (0, 2, 2)]
    chunks += [(t, 0, 4) for t in range(1, ntiles - 1) if t not in OFFLOAD]
    if (ntiles - 1) not in OFFLOAD:
        # tile 15 split 2+1+1: the final two single-group stores go down
        # different rings in parallel, each right after its own copy
        chunks += [
            (ntiles - 1, 0, 2),
            (ntiles - 1, 2, 1),
            (ntiles - 1, 3, 1),
        ]
    last = len(chunks) - 1

    with TileContext(nc) as tc:
        with (
            tc.tile_pool(name="wpool", bufs=1) as wpool,
            tc.tile_pool(name="xpool", bufs=12) as xpool,
            tc.tile_pool(name="vpool", bufs=4) as vpool,
            tc.tile_pool(name="opool", bufs=6) as opool,
            tc.tile_pool(name="pspool", bufs=8, space="PSUM") as pspool,
        ):
            # weight tile: rides the SP ring right behind tile 0's first
            # two groups (the baseline-measured fastest start: both sems
            # land ~10.7us).
            wsb = wpool.tile([128, 5, 128], F16, name="wsb")

            def wap(di):
                return wsb[:, di, :]

            oi = {t: i for i, t in enumerate(OFFLOAD)}
            ofst = {}

            def offload_front(t):
                """Load xof (sync ring, AFTER the first few x tiles -- a
                front-loaded xof steals cold-phase DMA capacity from the
                PE-pacing x loads) + GpSimd W-conv pair sums."""
                xoft = xpool.tile([128, SG, WP], F16, name="xof")
                nc.sync.dma_start(xoft[:], xo[oi[t]])
                t1 = vpool.tile([128, SG, W], F16, name="t1")
                t2 = vpool.tile([128, SG, W], F16, name="t2")
                nc.gpsimd.tensor_tensor(
                    t1[:], xoft[:, :, 0:W], xoft[:, :, 3 : 3 + W],
                    AluOpType.add,
                )
                nc.gpsimd.tensor_tensor(
                    t2[:], xoft[:, :, 1 : 1 + W], xoft[:, :, 2 : 2 + W],
                    AluOpType.add,
                )
                ofst[t] = (t1, t2)

            # HAM warm-up: a tiny [128,128] memset on DVE (~150ns at its
            # first post-preamble slot) unblocks a run of 128-col junk
            # matmuls that keep the PE busy from ~7.3us until the t0 DMA
            # lands (~9.9us) -- the clock-gate accumulator never resets.
            wjunk = wpool.tile([128, 128], F16, name="wjunk")
            nc.vector.memset(wjunk[:], 0.0)
            wscratch = wpool.tile([128, 4], F32, name="wscratch")
            wps = pspool.tile([128, 128], F32, name="wps", tag="ps")
            for r in range(WARMUP_MMS):
                nc.tensor.matmul(
                    wps[:],
                    wjunk[:],
                    wjunk[:],
                    start=(r == 0),
                    stop=(r == WARMUP_MMS - 1),
                )
            # warm store on the gpsimd software queue: scalar's instruction
            # stream must stay free for the x1 load issue (this store waits
            # on the junk chain and would block it until ~12.5us)
            nc.vector.tensor_copy(wscratch[:], wps[:, 0:4])
            nc.gpsimd.dma_start(warm_out, wscratch[:])

            ncopy = 0

            def offload_back(t, tail=False):
                """Fused h-band PE passes + copies + store, at stream end:
                ps = T^T t1 + (3T)^T t2 completes the separable blur."""
                t1, t2 = ofst[t]
                ot = opool.tile([128, SG, W], F16, name="ot")
                for q in range(4):
                    sl = slice(8 * q, 8 * (q + 1))
                    ps = pspool.tile([128, GQ * JB], F32, name="ps")
                    nc.tensor.matmul(
                        ps[:], wap(3), t1[:, sl, :], start=True, stop=False
                    )
                    nc.tensor.matmul(
                        ps[:], wap(4), t2[:, sl, :], start=False, stop=True
                    )
                    dst = ot[:, sl, :]
                    if q % 2 == 0:
                        nc.scalar.copy(dst, ps[:])
                    else:
                        nc.vector.tensor_copy(dst, ps[:])
                    if tail and q == 1:
                        nc.scalar.dma_start(
                            y[t][:, 0 : 2 * GQ * JB], ot[:, 0:16, :]
                        )
                if tail:
                    nc.sync.dma_start(
                        y[t][:, 2 * GQ * JB :], ot[:, 16:32, :]
                    )
                else:
                    nc.scalar.dma_start(y[t], ot[:])

            # tile 1 loads via the scalar ring, issued up-front (the scalar
            # engine's in-loop program position trails its copy stream, so
            # a late issue point makes loads just-in-time -- measured as a
            # 5.6us PE stall when tiles 12-14 were moved this way). One
            # 0.5MB transfer in parallel with the sync ring's cold ramp is
            # exactly the transfer the PE needs next; more would starve the
            # ramp (round-4 lesson).
            x1t = xpool.tile([128, 4 * GQ, JB], F16, name="xe")
            nc.scalar.dma_start(x1t[:], x[1])

            for ci, (dt, g0, ng) in enumerate(chunks):
                if dt == 1:
                    xt = x1t
                else:
                    xt = xpool.tile([128, ng * GQ, JB], F16, name="xt")
                    nc.sync.dma_start(
                        xt[:], x[dt][:, g0 * GF : (g0 + ng) * GF]
                    )
                if ci == 0:
                    # weights ride the SP ring second: land with chunk 0
                    nc.sync.dma_start(wsb[:], wm)
                grp = lambda g: xt[:, GQ * g : GQ * (g + 1), :]
                grpl = lambda g: xt[:, GQ * g : GQ * (g + 1), 0 : JB - 1]
                grpr = lambda g: xt[:, GQ * g : GQ * (g + 1), 1:JB]

                ot = opool.tile([128, ng * GQ, JB], F16, name="ot")
                # group-outer, d-inner: group q's PSUM is complete after its
                # own 3 matmuls; its copy runs while the PE streams q+1.
                # No padding: d=-1 skips output col jb=0, d=+1 skips jb=31.
                tailc = (not OFFLOAD) and ci >= last - 2
                for q in range(ng):
                    ps = pspool.tile([128, GQ, JB], F32, name="ps")
                    nc.tensor.matmul(
                        ps[:], wap(0), grp(q), start=True, stop=False
                    )
                    nc.tensor.matmul(
                        ps[:, :, 1:JB], wap(1), grpl(q), start=False, stop=False
                    )
                    nc.tensor.matmul(
                        ps[:, :, 0 : JB - 1], wap(2), grpr(q),
                        start=False, stop=True,
                    )
                    dst = ot[:, GQ * q : GQ * (q + 1), :]
                    if tailc:
                        # drain chunks: pin copy engines so the final two
                        # single-group chunks drain on scalar and vector
                        # (and then both rings) in parallel
                        if ci == last or (ci == last - 2 and q == 1):
                            nc.vector.tensor_copy(dst, ps[:])
                        else:
                            nc.scalar.copy(dst, ps[:])
                        continue
                    # alternate copy engine: DVE and ACT share the load
                    if ncopy % 2 == 0:
                        nc.vector.tensor_copy(dst, ps[:])
                    else:
                        nc.scalar.copy(dst, ps[:])
                    ncopy += 1
                ylo = g0 * GQ * JB
                if not tailc:
                    nc.scalar.dma_start(
                        y[dt][:, ylo : ylo + ng * GQ * JB], ot[:]
                    )
                else:
                    # drain stores: (15,0,2) and (15,3,1) ride the sync
                    # ring so scalar's chain is just copy(15b)+store(15b) --
                    # the two final stores overlap on different rings.
                    # (All-scalar draining was measured ~1.3us slower.)
                    eng = nc.scalar if ci == last - 1 else nc.sync
                    eng.dma_start(
                        y[dt][:, ylo : ylo + ng * GQ * JB], ot[:]
                    )
                # xof loads ride the sync ring behind x3..x6: early enough
                # for GpSimd's 28.8us of adds, late enough not to starve
                # the PE's own load stream during the cold ramp
                if OFFLOAD and ci - 4 in range(len(OFFLOAD)):
                    offload_front(OFFLOAD[ci - 4])

            # offloaded tiles' h-band passes close the PE stream
            for k, t in enumerate(OFFLOAD):
                offload_back(t, tail=(k == len(OFFLOAD) - 1))

    nc.compile()
    return nc


def get_nc(slices_per_core: int = SLICES_PER_CORE):
    if slices_per_core not in _NC_CACHE:
        _NC_CACHE[slices_per_core] = _build_nc(slices_per_core)
    return _NC_CACHE[slices_per_core]


def _pack_input(xs: np.ndarray):
    """[S, H, W] fp16 -> main tiles [S/64, 128, FREE] + offload tiles."""
    s = xs.shape[0]
    ntiles = s // TILE_SLICES
    v = np.empty((ntiles, 2, H, TILE_SLICES, JB), np.float16)
    xt = xs.reshape(ntiles, TILE_SLICES, H, W)
    v[:, 0] = xt[:, :, :, 0::2].transpose(0, 2, 1, 3)
    v[:, 1] = xt[:, :, :, 1::2].transpose(0, 2, 1, 3)
    xmain = np.ascontiguousarray(v.reshape(ntiles, 128, FREE))
    if not OFFLOAD:
        return xmain, None
    # offload tiles: partition (m, h), free (sg, w) with w zero-padded to 67
    xofs = np.zeros((len(OFFLOAD), 128, SG * WP), np.float16)
    for i, t in enumerate(OFFLOAD):
        xp = np.zeros((TILE_SLICES, H, WP), np.float16)
        xp[:, :, 2 : 2 + W] = xt[t]
        # (sg, m, h, w) -> (m, h, sg, w)
        xofs[i] = (
            xp.reshape(SG, 2, H, WP)
            .transpose(1, 2, 0, 3)
            .reshape(128, SG * WP)
        )
    return xmain, xofs


def _unpack_output(yp: np.ndarray) -> np.ndarray:
    """[S/64, 128, 64*JB] fp16 -> [S, H, W] fp16 (mixed per-tile layouts)."""
    ntiles = yp.shape[0]
    out = np.empty((ntiles, TILE_SLICES, H, W), np.float16)
    # main path: [jp, i, s, jb]
    v = yp.reshape(ntiles, 2, H, TILE_SLICES, JB)
    out[:, :, :, 0::2] = v[:, 0].transpose(0, 2, 1, 3)
    out[:, :, :, 1::2] = v[:, 1].transpose(0, 2, 1, 3)
    # offload path: [m, i, sg, w]
    for t in OFFLOAD:
        if t < ntiles:
            vo = yp[t].reshape(2, H, SG, W)
            out[t] = vo.transpose(2, 0, 1, 3).reshape(TILE_SLICES, H, W)
    return out.reshape(ntiles * TILE_SLICES, H, W)


def kernel(x: np.ndarray, kernel: np.ndarray, _trace: bool = False, **_tkw):
    xh = np.asarray(x).astype(np.float16)
    wmat = _build_wmat(kernel)
    b, c, h, w = x.shape
    xs = xh.reshape(b * c, h, w)
    spc = (b * c) // N_CORES
    nc = get_nc(spc)
    in_maps = []
    for k in range(N_CORES):
        xmain, xofs = _pack_input(xs[k * spc : (k + 1) * spc])
        m = {"x": xmain, "w": wmat}
        if xofs is not None:
            m["xo"] = xofs
        in_maps.append(m)
    res = run_bass_kernel_spmd(
        nc, in_maps, list(range(N_CORES)), trace=_trace, **_tkw
    )
    out = np.concatenate(
        [_unpack_output(res.results[k]["y"]) for k in range(N_CORES)], axis=0
    )
    result = out.reshape(b, c, h, w).astype(np.float32)
    if _trace:
        return result, res
    return result
